# revision 8
# baseline (speedup 1.0000x reference)
"""Trainium2 Bass kernel for nn_AwkwardRNNDoubleJagged.

The model is a 2-layer LSTM (width 512, scalar inputs) scanned sequentially
over 256 particles x feat_lens[p] timesteps, with an "event state" carry
(second half of h/c) chained across particles.  The computation is one strict
sequential chain of sum(feat_lens) LSTM-stack steps, so the kernel runs the
chain on one core with all weights resident in SBUF, skipping all masked
(t >= len) steps via a host-compacted schedule.

v2 design notes (from on-device microbenchmarks):
- values_load/snap + register-offset APs per *step* cost ~3.4us + ~80ns/matmul,
  so the step body uses only static APs and loop-index (ds(m,1)) addressing,
  which is free.
- a fully static unrolled program (256 peeled loops) costs ~12us/step extra in
  instruction-stream overhead (~3.4MB of instructions), so instead ONE shared
  step body is nested inside an outer particle loop: the inner For_i takes its
  [start, end) bounds from registers loaded per particle (~us-scale, amortized
  to ~0.1us/step).  The whole program is ~400 instructions and
  input-independent (bounds/xs are data), so it compiles once, ever.
- particle resets ([h_hi; 0] re-seed) are two DVE copies + two memsets per
  state tile in the outer loop body.
- gates (2048) live in PSUM as (128,16); gate blocks permuted [i,f,o,g] so one
  sigmoid covers cols 0-11 and one tanh cols 12-15.  P1 is accumulated by two
  column-interleaved matmul groups, so it is cleared by one zero-matmul over
  all 16 columns (per-column start=True corrupts neighbouring columns'
  partials at PSUM zero-region granularity -- measured 2e-2 drift).
- weights are bf16 lhsT tiles; h is bf16; cell state, biases and gate math are
  fp32.  fp8 weights measured *slower* (FWL off): bf16 23.8 ns/pair vs fp8
  57.8-145.6 ns/pair.
- final logits + log_softmax (10 outputs) are computed on host from the
  kernel's fp32 h1 readout.
"""
import functools
import numpy as np
import ml_dtypes

import concourse.bacc as bacc
import concourse.mybir as mybir
from concourse.bass import ds
from concourse.tile import TileContext
from concourse.bass_utils import run_bass_kernel_spmd

F32 = mybir.dt.float32
BF16 = mybir.dt.bfloat16
I32 = mybir.dt.int32

P_, F_, H_, OUT_ = 256, 128, 256, 10
HS = 2 * H_       # 512
G = 4 * HS        # 2048
NJ = 16
NK0 = 4
NK1 = 8
T_CAP = P_ * F_   # 32768
XCOLS = T_CAP + 1  # +1: inner-loop var bound is conservatively max(end)==T_CAP

SIG = mybir.ActivationFunctionType.Sigmoid
TANH = mybir.ActivationFunctionType.Tanh
MUL = mybir.AluOpType.mult
ADD = mybir.AluOpType.add


def _perm_gates(a):
    i, f, g, o = np.split(a, 4, axis=0)
    return np.concatenate([i, f, o, g], axis=0)


def _make_lhsT(Wp, nk):
    out = np.zeros((128, NJ * nk * 128), np.float32)
    for j in range(NJ):
        for k in range(nk):
            blk = Wp[128 * j:128 * (j + 1), 128 * k:128 * (k + 1)]
            out[:, (j * nk + k) * 128:(j * nk + k + 1) * 128] = blk.T
    return out


def _cols16(v):
    return v.reshape(NJ, 128).T.copy()


def _bounds(lens, n_steps=None):
    """int32 [1, 2*P_]: cols 2p/2p+1 = [start, end) of particle p's steps in
    the compacted stream; truncated schedules get start==end (loop skipped)."""
    total = sum(lens) if n_steps is None else min(n_steps, sum(lens))
    b = np.zeros((1, 2 * P_), np.int32)
    pos = 0
    for p, L in enumerate(lens):
        s = min(pos, total)
        e = min(pos + L, total)
        b[0, 2 * p] = s
        b[0, 2 * p + 1] = e
        pos += L
    return b


def _prep_host(inp):
    ev = np.asarray(inp["event"], np.float32)
    fl = np.asarray(inp["feat_lens"]).astype(np.int64)
    fl = np.maximum(fl, 1)

    xs = np.concatenate([ev[p, :fl[p]] for p in range(len(fl))]).astype(np.float32)
    T = int(fl.sum())
    xs_pad = np.zeros(XCOLS, np.float32)
    xs_pad[:T] = xs

    b0 = _perm_gates(np.asarray(inp["b_ih0"], np.float32) + np.asarray(inp["b_hh0"], np.float32))
    b1 = _perm_gates(np.asarray(inp["b_ih1"], np.float32) + np.asarray(inp["b_hh1"], np.float32))
    w_ih0 = _perm_gates(np.asarray(inp["w_ih0"], np.float32))[:, 0]
    W0p = _perm_gates(np.asarray(inp["w_hh0"], np.float32))
    W1full = np.concatenate(
        [_perm_gates(np.asarray(inp["w_ih1"], np.float32)),
         _perm_gates(np.asarray(inp["w_hh1"], np.float32))], axis=1)

    bf = ml_dtypes.bfloat16
    arrays = {
        "w0t": _make_lhsT(W0p, NK0).astype(bf),
        "w1t": _make_lhsT(W1full, NK1).astype(bf),
        "wi0c": _cols16(w_ih0),
        "b0c": _cols16(b0),
        "b1c": _cols16(b1),
        "xsb": np.ascontiguousarray(np.broadcast_to(xs_pad.astype(bf), (128, XCOLS))),
        "bnd": _bounds([int(v) for v in fl]),
    }
    return arrays, T, tuple(int(v) for v in fl)


def _build_nc():
    nc = bacc.Bacc(None)
    in_d = {
        "w0t": nc.dram_tensor("w0t", [128, NJ * NK0 * 128], BF16, kind="ExternalInput")[:],
        "w1t": nc.dram_tensor("w1t", [128, NJ * NK1 * 128], BF16, kind="ExternalInput")[:],
        "wi0c": nc.dram_tensor("wi0c", [128, 16], F32, kind="ExternalInput")[:],
        "b0c": nc.dram_tensor("b0c", [128, 16], F32, kind="ExternalInput")[:],
        "b1c": nc.dram_tensor("b1c", [128, 16], F32, kind="ExternalInput")[:],
        "xsb": nc.dram_tensor("xsb", [128, XCOLS], BF16, kind="ExternalInput")[:],
        "bnd": nc.dram_tensor("bnd", [1, 2 * P_], I32, kind="ExternalInput")[:],
    }
    hout_d = nc.dram_tensor("hout", [128, 16], F32, kind="ExternalOutput")

    with TileContext(nc) as tc:
        with tc.tile_pool(name="main", bufs=1) as pool:
            w0t = pool.tile([128, NJ * NK0 * 128], BF16)
            w1t = pool.tile([128, NJ * NK1 * 128], BF16)
            wi0c = pool.tile([128, 16], F32)
            b0c = pool.tile([128, 16], F32)
            b1c = pool.tile([128, 16], F32)
            xsb = pool.tile([128, XCOLS], BF16)
            bnd_t = pool.tile([1, 2 * P_], I32)

            h0s = pool.tile([128, 6], BF16)
            h1s = pool.tile([128, 6], BF16)
            c0s = pool.tile([128, 6], F32)
            c1s = pool.tile([128, 6], F32)
            xt0 = pool.tile([128, 16], F32)
            g0 = pool.tile([128, 16], F32)
            g1 = pool.tile([128, 16], F32)
            acts0 = pool.tile([128, 16], F32)
            acts1 = pool.tile([128, 16], F32)
            tc0 = pool.tile([128, 4], F32)
            tc1 = pool.tile([128, 4], F32)
            tma = pool.tile([128, 4], F32)
            tmb = pool.tile([128, 4], F32)
            tmc = pool.tile([128, 4], F32)
            tmd = pool.tile([128, 4], F32)
            hout = pool.tile([128, 16], F32)
            zl = pool.tile([1, 128], BF16)
            zr = pool.tile([1, 16], BF16)

            with tc.tile_pool(name="psum", bufs=1, space="PSUM") as pp:
                P0 = pp.tile([128, 16], F32)
                P1 = pp.tile([128, 16], F32)

                for name, tile in [("w0t", w0t), ("w1t", w1t), ("wi0c", wi0c),
                                   ("b0c", b0c), ("b1c", b1c), ("xsb", xsb),
                                   ("bnd", bnd_t)]:
                    nc.sync.dma_start(tile[:], in_d[name])
                for t in (h0s, h1s, c0s, c1s):
                    nc.vector.memset(t[:], 0.0)
                nc.vector.memset(zl[:], 0.0)
                nc.vector.memset(zr[:], 0.0)

                mm = functools.partial(nc.tensor.matmul, skip_group_check=True)
                act = nc.scalar.activation
                tt = nc.vector.tensor_tensor
                stt = nc.vector.scalar_tensor_tensor
                cp = nc.vector.tensor_copy

                def emit_step(i):
                    # layer 0
                    stt(xt0[:], wi0c[:], xsb[:, ds(i, 1)], b0c[:],
                        op0=MUL, op1=ADD)
                    for j in range(NJ):
                        for k in range(NK0):
                            mm(P0[:, j:j + 1],
                               w0t[:, (j * NK0 + k) * 128:(j * NK0 + k + 1) * 128],
                               h0s[:, k:k + 1],
                               start=(k == 0), stop=(k == NK0 - 1))
                    tt(g0[:], xt0[:], P0[:], op=ADD)
                    act(acts0[:, 0:12], g0[:, 0:12], SIG)
                    act(acts0[:, 12:16], g0[:, 12:16], TANH)
                    tt(tma[:], acts0[:, 0:4], acts0[:, 12:16], op=MUL)
                    tt(tmb[:], acts0[:, 4:8], c0s[:, 0:4], op=MUL)
                    tt(c0s[:, 0:4], tma[:], tmb[:], op=ADD)
                    act(tc0[:], c0s[:, 0:4], TANH)
                    tt(h0s[:, 0:4], acts0[:, 8:12], tc0[:], op=MUL)
                    # layer 1: recurrent part first (old h1), then update (new
                    # h0).  P1's two groups interleave by column, so clear the
                    # whole tile with one zero matmul (see module docstring).
                    mm(P1[:, 0:16], zl[:, :], zr[:, :], start=True, stop=False)
                    for j in range(NJ):
                        for k in range(4):
                            mm(P1[:, j:j + 1],
                               w1t[:, (j * NK1 + 4 + k) * 128:(j * NK1 + 5 + k) * 128],
                               h1s[:, k:k + 1],
                               start=False, stop=False)
                    for j in range(NJ):
                        for k in range(4):
                            mm(P1[:, j:j + 1],
                               w1t[:, (j * NK1 + k) * 128:(j * NK1 + k + 1) * 128],
                               h0s[:, k:k + 1],
                               start=False, stop=(k == 3))
                    tt(g1[:], b1c[:], P1[:], op=ADD)
                    act(acts1[:, 0:12], g1[:, 0:12], SIG)
                    act(acts1[:, 12:16], g1[:, 12:16], TANH)
                    tt(tmc[:], acts1[:, 0:4], acts1[:, 12:16], op=MUL)
                    tt(tmd[:], acts1[:, 4:8], c1s[:, 0:4], op=MUL)
                    tt(c1s[:, 0:4], tmc[:], tmd[:], op=ADD)
                    act(tc1[:], c1s[:, 0:4], TANH)
                    tt(h1s[:, 0:4], acts1[:, 8:12], tc1[:], op=MUL)

                with tc.For_i(0, P_, 1) as p:
                    # reset: state <- [state_hi ; 0] (harmless no-op for p=0,
                    # where the state is all zero)
                    for t in (h0s, h1s, c0s, c1s):
                        cp(t[:, 0:2], t[:, 2:4])
                        nc.vector.memset(t[:, 2:4], 0.0)
                    s = nc.values_load(bnd_t[0:1, ds(2 * p, 1)],
                                       min_val=0, max_val=T_CAP,
                                       skip_runtime_bounds_check=True)
                    e = nc.values_load(bnd_t[0:1, ds(2 * p + 1, 1)],
                                       min_val=0, max_val=T_CAP,
                                       skip_runtime_bounds_check=True)
                    with tc.For_i(s, e, 1) as m:
                        emit_step(m)

                tt(hout[:, 0:4], acts1[:, 8:12], tc1[:], op=MUL)
                tt(hout[:, 4:8], acts0[:, 8:12], tc0[:], op=MUL)
                cp(hout[:, 8:12], c0s[:, 0:4])
                cp(hout[:, 12:16], c1s[:, 0:4])
                nc.sync.dma_start(hout_d[:], hout[:])

    nc.finalize()
    return nc


_CACHE = {}


def kernel(**inputs) -> np.ndarray:
    arrays, T, lens = _prep_host(inputs)

    if "nc" not in _CACHE:
        _CACHE["nc"] = _build_nc()
    nc = _CACHE["nc"]

    # The chain is strictly sequential (each step's GEMVs consume the previous
    # step's hidden state, particles are chained through the event state):
    # there is no cross-core parallelism to exploit, so run on core 0 only —
    # SPMD copies on the other 7 cores would just 8x the upload payload.
    res = run_bass_kernel_spmd(nc, [arrays], core_ids=[0])
    hout = res.results[0]["hout"]
    h1 = hout[:, 0:4].T.reshape(-1).astype(np.float64)   # (512,) final top-layer h

    w_out = np.asarray(inputs["w_out"], np.float64)
    b_out = np.asarray(inputs["b_out"], np.float64)
    logits = h1 @ w_out.T + b_out
    ls = logits - np.log(np.exp(logits - logits.max()).sum()) - logits.max()
    return ls[None, :].astype(np.float32)


# revision 9
# speedup vs baseline: 1.0860x; 1.0860x over previous
"""Trainium2 Bass kernel for nn_AwkwardRNNDoubleJagged.

The model is a 2-layer LSTM (width 512, scalar inputs) scanned sequentially
over 256 particles x feat_lens[p] timesteps, with an "event state" carry
(second half of h/c) chained across particles.  The computation is one strict
sequential chain of sum(feat_lens) LSTM-stack steps, so the kernel runs the
chain on one core with all weights resident in SBUF, skipping all masked
(t >= len) steps via a host-compacted schedule.

v2 design notes (from on-device microbenchmarks):
- values_load/snap + register-offset APs per *step* cost ~3.4us + ~80ns/matmul,
  so the step body uses only static APs and loop-index (ds(m,1)) addressing,
  which is free.
- a fully static unrolled program (256 peeled loops) costs ~12us/step extra in
  instruction-stream overhead (~3.4MB of instructions), so instead ONE shared
  step body is nested inside an outer particle loop: the inner For_i takes its
  [start, end) bounds from registers loaded per particle (~us-scale, amortized
  to ~0.1us/step).  The whole program is ~400 instructions and
  input-independent (bounds/xs are data), so it compiles once, ever.
- particle resets ([h_hi; 0] re-seed) are two DVE copies + two memsets per
  state tile in the outer loop body.
- gates (2048) live in PSUM as (128,16); gate blocks permuted [i,f,o,g] so one
  sigmoid covers cols 0-11 and one tanh cols 12-15.  P1 is accumulated by two
  column-interleaved matmul groups, so it is cleared by one zero-matmul over
  all 16 columns (per-column start=True corrupts neighbouring columns'
  partials at PSUM zero-region granularity -- measured 2e-2 drift).
- weights are bf16 lhsT tiles; h is bf16; cell state, biases and gate math are
  fp32.  fp8 weights measured *slower* (FWL off): bf16 23.8 ns/pair vs fp8
  57.8-145.6 ns/pair.
- final logits + log_softmax (10 outputs) are computed on host from the
  kernel's fp32 h1 readout.
"""
import functools
import numpy as np
import ml_dtypes

import concourse.bacc as bacc
import concourse.mybir as mybir
from concourse.bass import ds
from concourse.tile import TileContext
from concourse.bass_utils import run_bass_kernel_spmd

F32 = mybir.dt.float32
BF16 = mybir.dt.bfloat16
I32 = mybir.dt.int32

P_, F_, H_, OUT_ = 256, 128, 256, 10
HS = 2 * H_       # 512
G = 4 * HS        # 2048
NJ = 16
NK0 = 4
NK1 = 8
T_CAP = P_ * F_   # 32768
XCOLS = T_CAP + 2  # +2: 2-step body reads ds(m+1,1); m's conservative max is T_CAP

SIG = mybir.ActivationFunctionType.Sigmoid
TANH = mybir.ActivationFunctionType.Tanh
MUL = mybir.AluOpType.mult
ADD = mybir.AluOpType.add


def _perm_gates(a):
    i, f, g, o = np.split(a, 4, axis=0)
    return np.concatenate([i, f, o, g], axis=0)


def _make_lhsT(Wp, nk):
    out = np.zeros((128, NJ * nk * 128), np.float32)
    for j in range(NJ):
        for k in range(nk):
            blk = Wp[128 * j:128 * (j + 1), 128 * k:128 * (k + 1)]
            out[:, (j * nk + k) * 128:(j * nk + k + 1) * 128] = blk.T
    return out


def _cols16(v):
    return v.reshape(NJ, 128).T.copy()


def _bounds(lens, n_steps=None):
    """int32 [1, 3*P_]: cols 3p..3p+2 = [start, even_end, end) of particle p's
    steps in the compacted stream.  The main loop runs [start, even_end) two
    steps per iteration; a 0/1-trip tail loop runs [even_end, end).  Truncated
    schedules get start==even_end==end (loops skipped)."""
    total = sum(lens) if n_steps is None else min(n_steps, sum(lens))
    b = np.zeros((1, 3 * P_), np.int32)
    pos = 0
    for p, L in enumerate(lens):
        s = min(pos, total)
        e = min(pos + L, total)
        b[0, 3 * p] = s
        b[0, 3 * p + 1] = s + 2 * ((e - s) // 2)
        b[0, 3 * p + 2] = e
        pos += L
    return b


def _prep_host(inp):
    ev = np.asarray(inp["event"], np.float32)
    fl = np.asarray(inp["feat_lens"]).astype(np.int64)
    fl = np.maximum(fl, 1)

    xs = np.concatenate([ev[p, :fl[p]] for p in range(len(fl))]).astype(np.float32)
    T = int(fl.sum())
    xs_pad = np.zeros(XCOLS, np.float32)
    xs_pad[:T] = xs

    b0 = _perm_gates(np.asarray(inp["b_ih0"], np.float32) + np.asarray(inp["b_hh0"], np.float32))
    b1 = _perm_gates(np.asarray(inp["b_ih1"], np.float32) + np.asarray(inp["b_hh1"], np.float32))
    w_ih0 = _perm_gates(np.asarray(inp["w_ih0"], np.float32))[:, 0]
    W0p = _perm_gates(np.asarray(inp["w_hh0"], np.float32))
    W1full = np.concatenate(
        [_perm_gates(np.asarray(inp["w_ih1"], np.float32)),
         _perm_gates(np.asarray(inp["w_hh1"], np.float32))], axis=1)

    bf = ml_dtypes.bfloat16
    arrays = {
        "w0t": _make_lhsT(W0p, NK0).astype(bf),
        "w1t": _make_lhsT(W1full, NK1).astype(bf),
        "wi0c": _cols16(w_ih0),
        "b0c": _cols16(b0),
        "b1c": _cols16(b1),
        "xsb": np.ascontiguousarray(np.broadcast_to(xs_pad.astype(bf), (128, XCOLS))),
        "bnd": _bounds([int(v) for v in fl]),
    }
    return arrays, T, tuple(int(v) for v in fl)


def _build_nc():
    nc = bacc.Bacc(None)
    in_d = {
        "w0t": nc.dram_tensor("w0t", [128, NJ * NK0 * 128], BF16, kind="ExternalInput")[:],
        "w1t": nc.dram_tensor("w1t", [128, NJ * NK1 * 128], BF16, kind="ExternalInput")[:],
        "wi0c": nc.dram_tensor("wi0c", [128, 16], F32, kind="ExternalInput")[:],
        "b0c": nc.dram_tensor("b0c", [128, 16], F32, kind="ExternalInput")[:],
        "b1c": nc.dram_tensor("b1c", [128, 16], F32, kind="ExternalInput")[:],
        "xsb": nc.dram_tensor("xsb", [128, XCOLS], BF16, kind="ExternalInput")[:],
        "bnd": nc.dram_tensor("bnd", [1, 3 * P_], I32, kind="ExternalInput")[:],
    }
    hout_d = nc.dram_tensor("hout", [128, 16], F32, kind="ExternalOutput")

    with TileContext(nc) as tc:
        with tc.tile_pool(name="main", bufs=1) as pool:
            w0t = pool.tile([128, NJ * NK0 * 128], BF16)
            w1t = pool.tile([128, NJ * NK1 * 128], BF16)
            wi0c = pool.tile([128, 16], F32)
            b0c = pool.tile([128, 16], F32)
            b1c = pool.tile([128, 16], F32)
            xsb = pool.tile([128, XCOLS], BF16)
            bnd_t = pool.tile([1, 3 * P_], I32)

            h0s = pool.tile([128, 6], BF16)
            h1s = pool.tile([128, 6], BF16)
            c0s = pool.tile([128, 6], F32)
            c1s = pool.tile([128, 6], F32)
            xt0 = pool.tile([128, 16], F32)
            g0 = pool.tile([128, 16], F32)
            g1 = pool.tile([128, 16], F32)
            acts0 = pool.tile([128, 16], F32)
            acts1 = pool.tile([128, 16], F32)
            tc0 = pool.tile([128, 4], F32)
            tc1 = pool.tile([128, 4], F32)
            tma = pool.tile([128, 4], F32)
            tmb = pool.tile([128, 4], F32)
            tmc = pool.tile([128, 4], F32)
            tmd = pool.tile([128, 4], F32)
            hout = pool.tile([128, 16], F32)
            zl = pool.tile([1, 128], BF16)
            zr = pool.tile([1, 16], BF16)

            with tc.tile_pool(name="psum", bufs=1, space="PSUM") as pp:
                P0 = pp.tile([128, 16], F32)
                P1 = pp.tile([128, 16], F32)

                for name, tile in [("w0t", w0t), ("w1t", w1t), ("wi0c", wi0c),
                                   ("b0c", b0c), ("b1c", b1c), ("xsb", xsb),
                                   ("bnd", bnd_t)]:
                    nc.sync.dma_start(tile[:], in_d[name])
                for t in (h0s, h1s, c0s, c1s):
                    nc.vector.memset(t[:], 0.0)
                nc.vector.memset(zl[:], 0.0)
                nc.vector.memset(zr[:], 0.0)

                mm = functools.partial(nc.tensor.matmul, skip_group_check=True)
                act = nc.scalar.activation
                tt = nc.vector.tensor_tensor
                stt = nc.vector.scalar_tensor_tensor
                cp = nc.vector.tensor_copy

                def emit_step(i):
                    # layer 0
                    stt(xt0[:], wi0c[:], xsb[:, ds(i, 1)], b0c[:],
                        op0=MUL, op1=ADD)
                    for j in range(NJ):
                        for k in range(NK0):
                            mm(P0[:, j:j + 1],
                               w0t[:, (j * NK0 + k) * 128:(j * NK0 + k + 1) * 128],
                               h0s[:, k:k + 1],
                               start=(k == 0), stop=(k == NK0 - 1))
                    tt(g0[:], xt0[:], P0[:], op=ADD)
                    act(acts0[:, 0:12], g0[:, 0:12], SIG)
                    act(acts0[:, 12:16], g0[:, 12:16], TANH)
                    tt(tma[:], acts0[:, 0:4], acts0[:, 12:16], op=MUL)
                    tt(tmb[:], acts0[:, 4:8], c0s[:, 0:4], op=MUL)
                    tt(c0s[:, 0:4], tma[:], tmb[:], op=ADD)
                    act(tc0[:], c0s[:, 0:4], TANH)
                    tt(h0s[:, 0:4], acts0[:, 8:12], tc0[:], op=MUL)
                    # layer 1: recurrent part first (old h1), then update (new
                    # h0).  P1's two groups interleave by column, so clear the
                    # whole tile with one zero matmul (see module docstring).
                    mm(P1[:, 0:16], zl[:, :], zr[:, :], start=True, stop=False)
                    for j in range(NJ):
                        for k in range(4):
                            mm(P1[:, j:j + 1],
                               w1t[:, (j * NK1 + 4 + k) * 128:(j * NK1 + 5 + k) * 128],
                               h1s[:, k:k + 1],
                               start=False, stop=False)
                    for j in range(NJ):
                        for k in range(4):
                            mm(P1[:, j:j + 1],
                               w1t[:, (j * NK1 + k) * 128:(j * NK1 + k + 1) * 128],
                               h0s[:, k:k + 1],
                               start=False, stop=(k == 3))
                    tt(g1[:], b1c[:], P1[:], op=ADD)
                    act(acts1[:, 0:12], g1[:, 0:12], SIG)
                    act(acts1[:, 12:16], g1[:, 12:16], TANH)
                    tt(tmc[:], acts1[:, 0:4], acts1[:, 12:16], op=MUL)
                    tt(tmd[:], acts1[:, 4:8], c1s[:, 0:4], op=MUL)
                    tt(c1s[:, 0:4], tmc[:], tmd[:], op=ADD)
                    act(tc1[:], c1s[:, 0:4], TANH)
                    tt(h1s[:, 0:4], acts1[:, 8:12], tc1[:], op=MUL)

                with tc.For_i(0, P_, 1) as p:
                    # reset: state <- [state_hi ; 0] (harmless no-op for p=0,
                    # where the state is all zero)
                    for t in (h0s, h1s, c0s, c1s):
                        cp(t[:, 0:2], t[:, 2:4])
                        nc.vector.memset(t[:, 2:4], 0.0)
                    s = nc.values_load(bnd_t[0:1, ds(3 * p, 1)],
                                       min_val=0, max_val=T_CAP,
                                       skip_runtime_bounds_check=True)
                    e2 = nc.values_load(bnd_t[0:1, ds(3 * p + 1, 1)],
                                        min_val=0, max_val=T_CAP,
                                        skip_runtime_bounds_check=True)
                    e = nc.values_load(bnd_t[0:1, ds(3 * p + 2, 1)],
                                       min_val=0, max_val=T_CAP,
                                       skip_runtime_bounds_check=True)
                    # 2-step body: PE stays busy ~7.8us per iteration so the
                    # HAM activity window never sees a >3.4us idle gap (a
                    # 1-step body exposes elem1+loop barrier every step and
                    # the PE re-throttles to 1.2 GHz).
                    with tc.For_i(s, e2, 2) as m:
                        emit_step(m)
                        emit_step(m + 1)
                    with tc.For_i(e2, e, 1) as m:
                        emit_step(m)

                tt(hout[:, 0:4], acts1[:, 8:12], tc1[:], op=MUL)
                tt(hout[:, 4:8], acts0[:, 8:12], tc0[:], op=MUL)
                cp(hout[:, 8:12], c0s[:, 0:4])
                cp(hout[:, 12:16], c1s[:, 0:4])
                nc.sync.dma_start(hout_d[:], hout[:])

    nc.finalize()
    return nc


_CACHE = {}


def kernel(**inputs) -> np.ndarray:
    arrays, T, lens = _prep_host(inputs)

    if "nc" not in _CACHE:
        _CACHE["nc"] = _build_nc()
    nc = _CACHE["nc"]

    # The chain is strictly sequential (each step's GEMVs consume the previous
    # step's hidden state, particles are chained through the event state):
    # there is no cross-core parallelism to exploit, so run on core 0 only —
    # SPMD copies on the other 7 cores would just 8x the upload payload.
    res = run_bass_kernel_spmd(nc, [arrays], core_ids=[0])
    hout = res.results[0]["hout"]
    h1 = hout[:, 0:4].T.reshape(-1).astype(np.float64)   # (512,) final top-layer h

    w_out = np.asarray(inputs["w_out"], np.float64)
    b_out = np.asarray(inputs["b_out"], np.float64)
    logits = h1 @ w_out.T + b_out
    ls = logits - np.log(np.exp(logits - logits.max()).sum()) - logits.max()
    return ls[None, :].astype(np.float32)
